# revision 11
# baseline (speedup 1.0000x reference)
"""DivisiveNormBlock kernel v4 for 8 Trainium2 NeuronCores.

out[b,i] = x[b,i]^nU[i] / (bias[i]^nU[i] + sum_u conv2d(x[b,i]^nI[i,u], g[i,u]))

v4 strategy: G=16 images per group with KB=8 basis nodes per channel
(GKB=128) -> only NGR=2 groups, halving both the conv matmul passes
(NGR*T2 = 12) and the Act-engine exp volume vs v2. Bias is folded into
the tap-sum matmul via a constant-ones partition row (K=97), the
reciprocal runs straight from PSUM, and the final numerator/multiply
work in the compact [128, 784] quarter-major layout. Weight DMAs and
border memsets are hoisted out of the steady-state loop; group 1's
basis passes interleave into group 0's conv to keep the PE p-state
warm. The timing loop unrolls UNROLL bodies per hardware iteration
with double-buffered head/tail tiles so consecutive iterations
software-pipeline across the loop.
"""

import math
import numpy as np
import ml_dtypes

C = 128
S = 56
KS = 6
N_CORES = 8
IL = C // N_CORES          # 16 channels per core
NBI = IL * 2               # 32 (channel, batch) images per core
WP = 64                    # padded image row pitch
IMG = 64 * WP              # 4096
SOUT = S * WP              # 3584
W2Z = 3648                 # z2 width (SOUT + 5 rounded to 64)
T1 = 6                     # residual taps (kx in {0..5})
T2 = 6                     # accumulated row-shift passes (ky in {0..5})
NEG = -1e30

KB = 8                     # basis nodes per channel
G = 16                     # images per conv group
NGR = NBI // G             # 2
GKB = G * KB               # 128
T1G = T1 * G               # 96
UNROLL = 1                 # bodies per hardware loop iteration

_cache = {}


def _gaussian_bank(theta, p, sig, a):
    K = 3
    coords = np.linspace(-K, K, 2 * K)
    xv, yv = np.meshgrid(coords, coords, indexing="ij")
    ct = np.cos(theta)[:, :, None, None]
    st = np.sin(theta)[:, :, None, None]
    xr = xv * ct + yv * st
    yr = -xv * st + yv * ct
    p2 = (p ** 2)[:, :, None, None]
    s2 = (sig ** 2)[:, :, None, None]
    amp = (a / (2.0 * np.pi * p * sig))[:, :, None, None]
    return amp * np.exp(-0.5 * (xr ** 2 / p2 + yr ** 2 / s2))   # [C,C,6,6]


def _fit_chan(nvals, kb):
    """Per-channel LS fit: e^{n l} ~ sum_k c_k e^{a_k l}, l in [-19, 0]."""
    n_lo = max(float(nvals.min()) * 0.9, 1e-4)
    n_hi = float(nvals.max()) * 1.02
    aks = np.geomspace(n_lo, n_hi, kb)
    l_grid = np.linspace(-19.0, 0.0, 4000)
    A = np.exp(np.outer(l_grid, aks))
    AtA = A.T @ A + 1e-6 * np.eye(kb)
    Y = np.exp(np.outer(l_grid, nvals))
    Cf = np.linalg.solve(AtA, A.T @ Y)               # [kb, C]
    return aks, Cf


def _build_host_params(theta, p, sig, a, nI, nU, bias, kb=KB):
    f64 = np.float64
    g = _gaussian_bank(theta.astype(f64), p.astype(f64), sig.astype(f64),
                       a.astype(f64))                 # [C,C,6,6]
    nI64 = nI.astype(f64)
    aks_all = np.zeros((C, kb))
    W2_all = np.zeros((C, kb, KS, KS))
    for i in range(C):
        aks, Cf = _fit_chan(nI64[i], kb)
        aks_all[i] = aks
        W2_all[i] = np.einsum("uyx,ku->kyx", g[i], Cf)
    biasP = bias.astype(f64) ** nU.astype(f64)
    return aks_all, W2_all, biasP


def _build_program(loop_n=None, debug=False, unroll_n=None):
    import concourse.bacc as bacc
    import concourse.mybir as mybir
    from concourse.tile import TileContext
    from contextlib import nullcontext

    f32, f32r, bf16 = mybir.dt.float32, mybir.dt.float32r, mybir.dt.bfloat16
    AF = mybir.ActivationFunctionType

    # bodies per hw-loop iteration: loop_n=k*UNROLL runs For_i(k) x UNROLL
    if unroll_n is not None:
        hw_n, nbody = None, unroll_n
    elif loop_n and loop_n % UNROLL == 0:
        hw_n, nbody = loop_n // UNROLL, UNROLL
    elif loop_n:
        hw_n, nbody = loop_n, 1
    else:
        hw_n, nbody = None, 1

    nc = bacc.Bacc("TRN2", debug=False)
    xs = nc.dram_tensor("xs", [128, 784], f32, kind="ExternalInput")
    e3 = nc.dram_tensor("e3", [NBI, NGR * GKB], f32r, kind="ExternalInput")
    w3 = nc.dram_tensor("w3", [GKB, NGR * T2 * T1G], f32r, kind="ExternalInput")
    o3 = nc.dram_tensor("o3", [T1G + 1, NGR * NBI], bf16, kind="ExternalInput")
    nUr = nc.dram_tensor("nUr", [128, 1], f32, kind="ExternalInput")
    y = nc.dram_tensor("y", [128, 784], f32, kind="ExternalOutput")

    # conv output chunks over z2 cols [0, W2Z): 7 x 512 + 1 x 64
    conv_chunks = [(512 * i, 512) for i in range(7)] + [(3584, W2Z - 3584)]

    with TileContext(nc) as tc:
        with tc.tile_pool(name="const", bufs=1) as cpool, \
             tc.tile_pool(name="pbc", bufs=2, space="PSUM") as pbc, \
             tc.tile_pool(name="pcv", bufs=2, space="PSUM") as pcv, \
             tc.tile_pool(name="pts", bufs=4, space="PSUM") as pts:
            # ---- persistent tiles; head/tail set double-buffered ----
            x_t = [cpool.tile([128, 784], f32, name=f"x_{d}") for d in (0, 1)]
            l_t = [cpool.tile([128, 784], f32, name=f"l_{d}") for d in (0, 1)]
            num_t = [cpool.tile([128, 784], f32, name=f"num_{d}")
                     for d in (0, 1)]
            rT_t = [cpool.tile([128, 784], f32, name=f"rT_{d}")
                    for d in (0, 1)]
            o_t = [cpool.tile([128, 784], f32, name=f"o_{d}") for d in (0, 1)]
            r_t = [cpool.tile([NBI, SOUT], f32, name=f"r_{d}") for d in (0, 1)]
            lp_t = cpool.tile([NBI, IMG], f32r)
            b3_t = [cpool.tile([GKB, IMG], f32r, name=f"b3_{g}")
                    for g in range(NGR)]
            z2_t = [cpool.tile([T1G, W2Z], bf16, name=f"z2_{g}")
                    for g in range(NGR)]
            z3_t = [cpool.tile([T1G + 1, SOUT], bf16, name=f"z3_{g}")
                    for g in range(NGR)]
            e3_t = cpool.tile([NBI, NGR * GKB], f32r)
            w3_t = cpool.tile([GKB, NGR * T2 * T1G], f32r)
            o3_t = cpool.tile([T1G + 1, NGR * NBI], bf16)
            nU_t = cpool.tile([128, 1], f32)

            # ---- hoisted setup: params + constant borders (once) ----
            nc.sync.dma_start(e3_t[:], e3.ap())
            nc.sync.dma_start(w3_t[:], w3.ap())
            nc.sync.dma_start(o3_t[:], o3.ap())
            nc.sync.dma_start(nU_t[:], nUr.ap())
            nc.vector.memset(lp_t[:].bitcast(f32), NEG)
            # ones row for the bias fold (partition 96 of group-0 z3)
            nc.vector.memset(z3_t[0][T1G:T1G + 1, :], 1.0)

            def body(pp):
                xt, lt, numt = x_t[pp], l_t[pp], num_t[pp]
                rt, rTt, ot = r_t[pp], rT_t[pp], o_t[pp]
                nc.sync.dma_start(xt[:], xs.ap())

                # l = clamp(ln(x)); x=0 -> -inf -> -1e30
                nc.scalar.activation(lt[:], xt[:], AF.Ln)
                nc.vector.tensor_scalar_max(lt[:], lt[:], NEG)

                # padded log images, one partition per (i,b); 4 fused DMAs
                pad_engs = (nc.sync, nc.gpsimd, nc.sync, nc.gpsimd)
                for q in range(4):
                    src = lt[32 * q:32 * q + 32, :].bitcast(f32r).rearrange(
                        "p (r c) -> p r c", c=56)
                    dst = lp_t[:].rearrange("p (r c) -> p r c", r=64)[
                        :, 2 + 14 * q:2 + 14 * q + 14, 2:58]
                    pad_engs[q].dma_start(dst, src)

                def b3_pass(g, h):
                    pb = pbc.tile([GKB, 512], f32, tag="pb")
                    nc.tensor.matmul(
                        pb[:, :],
                        e3_t[:, GKB * g:GKB * g + GKB],
                        lp_t[:, 512 * h:512 * h + 512],
                        start=True, stop=True)
                    nc.scalar.activation(
                        b3_t[g][:, 512 * h:512 * h + 512], pb[:, :], AF.Exp)

                # group 0 basis maps: b3[im*KB+k,s] = exp(aks[i_im,k]*lp[im,s])
                for h in range(IMG // 512):
                    b3_pass(0, h)

                # conv with ky-shifted reads accumulated over T2 passes:
                # z2[im*T1 + t1, s] = sum_k sum_t2 W2[i_im,k,t2,t1]
                #                     * b3[im*KB+k, s + 64*t2]
                nmix = 0

                def conv_group(g, interleave, lo=0, hi=8):
                    nonlocal nmix
                    for ci, (s0, cw) in list(enumerate(conv_chunks))[lo:hi]:
                        pc = pcv.tile([T1G, 512], f32, tag="pc")
                        for t2 in range(T2):
                            nc.tensor.matmul(
                                pc[:, 0:cw],
                                w3_t[:, T1G * (T2 * g + t2):
                                     T1G * (T2 * g + t2) + T1G],
                                b3_t[g][:, s0 + 64 * t2:s0 + 64 * t2 + cw],
                                start=(t2 == 0), stop=(t2 == T2 - 1))
                        if nmix % 2 == 0:
                            nc.vector.tensor_copy(z2_t[g][:, s0:s0 + cw],
                                                  pc[:, 0:cw])
                        else:
                            nc.scalar.copy(z2_t[g][:, s0:s0 + cw],
                                           pc[:, 0:cw])
                        nmix += 1
                        if interleave and ci < IMG // 512:
                            b3_pass(1, ci)

                def realign(g, c0=0, c1=SOUT):
                    # residual kx realign: 6 strided-partition DMAs
                    re_engs = (nc.sync, nc.gpsimd, nc.sync,
                               nc.gpsimd, nc.sync, nc.gpsimd)
                    for t1 in range(T1):
                        re_engs[t1].dma_start(
                            z3_t[g][t1:T1G:T1, c0:c1],
                            z2_t[g][t1:T1G:T1, t1 + c0:t1 + c1])

                def tap(ch):
                    # tap sum over groups (o3 cols select the group's
                    # images; row 96 of group 0 adds bias via the ones
                    # row), then the reciprocal straight from PSUM
                    sl = slice(512 * ch, 512 * ch + 512)
                    pt = pts.tile([NBI, 512], f32, tag="pt")
                    nc.tensor.matmul(
                        pt[:, :], o3_t[0:T1G + 1, 0:NBI],
                        z3_t[0][0:T1G + 1, sl],
                        start=True, stop=False, skip_group_check=True)
                    nc.tensor.matmul(
                        pt[:, :], o3_t[0:T1G, NBI:2 * NBI],
                        z3_t[1][0:T1G, sl],
                        start=False, stop=True, skip_group_check=True)
                    nc.vector.reciprocal(rt[:, sl], pt[:, :])

                tr_engs = (nc.sync, nc.gpsimd, nc.sync, nc.gpsimd)

                def transpose_q(q):
                    # inverse-pad DMA back to [128, 784] quarter-major
                    src = rt[:].rearrange("p (r c) -> p r c", c=64)[
                        :, 14 * q:14 * q + 14, 0:56]
                    dst = rTt[32 * q:32 * q + 32, :].rearrange(
                        "p (r c) -> p r c", c=56)
                    tr_engs[q].dma_start(dst, src)

                conv_group(0, interleave=True)
                realign(0)
                # numerator x^nU = exp(nU * l) on the compact layout
                nc.scalar.activation(numt[:], lt[:], AF.Exp, scale=nU_t[:])
                conv_group(1, interleave=False)
                realign(1)
                for ch in range(SOUT // 512):
                    tap(ch)
                for q in range(4):
                    transpose_q(q)
                nc.vector.tensor_mul(ot[:], numt[:], rTt[:])
                nc.sync.dma_start(y.ap(), ot[:])

            loop_ctx = tc.For_i(0, hw_n, 1) if hw_n else nullcontext()
            with loop_ctx:
                for it in range(nbody):
                    body(it % 2)

    nc.compile()
    return nc


def _get_compiled(theta, p, sig, a, nI, nU, bias):
    key = "prog"
    if key in _cache:
        return _cache[key]
    aks_all, W2_all, biasP = _build_host_params(theta, p, sig, a, nI, nU, bias)
    nc = _build_program()

    bft = ml_dtypes.bfloat16
    core_ins = []
    for c in range(N_CORES):
        i0 = IL * c
        e3 = np.zeros((NBI, NGR * GKB), np.float32)
        w3 = np.zeros((GKB, NGR * T2 * T1G), np.float32)
        o3 = np.zeros((T1G + 1, NGR * NBI), np.float32)
        for g in range(NGR):
            for im in range(G):
                bi = G * g + im
                i = i0 + bi // 2
                e3[bi, GKB * g + KB * im:GKB * g + KB * im + KB] = aks_all[i]
                for t2 in range(T2):
                    blk = T1G * (T2 * g + t2)
                    for t1 in range(T1):
                        w3[KB * im:KB * im + KB, blk + T1 * im + t1] = \
                            W2_all[i, :, t2, t1]
                o3[T1 * im:T1 * im + T1, NBI * g + bi] = 1.0
        for bi in range(NBI):
            o3[T1G, bi] = biasP[i0 + bi // 2]     # bias via group-0 ones row
        nU_rep = np.repeat(nU[i0:i0 + IL].astype(np.float32), 2)
        nU128 = np.tile(nU_rep, 4)[:, None]       # quarter-major partitions
        core_ins.append({
            "e3": np.ascontiguousarray(e3),
            "w3": np.ascontiguousarray(w3),
            "o3": np.ascontiguousarray(o3.astype(bft)),
            "nUr": np.ascontiguousarray(nU128),
        })
    _cache[key] = (nc, core_ins)
    return _cache[key]


def make_in_maps(x, core_ins):
    in_maps = []
    for c in range(N_CORES):
        i0 = IL * c
        xc = np.transpose(x[:, i0:i0 + IL], (1, 0, 2, 3))   # [16, 2, 56, 56]
        # row = q*32 + bi: quarter-major so each 14-row slab of every image
        # sits in one contiguous 32-partition block
        xs = xc.reshape(IL * 2, 4, 784).transpose(1, 0, 2).reshape(128, 784)
        in_maps.append({"xs": np.ascontiguousarray(xs.astype(np.float32)),
                        **core_ins[c]})
    return in_maps


def kernel(x, theta, p, sig, a, nI, nU, bias):
    from concourse import bass_utils

    x = np.asarray(x)
    nc, core_ins = _get_compiled(
        np.asarray(theta), np.asarray(p), np.asarray(sig), np.asarray(a),
        np.asarray(nI), np.asarray(nU), np.asarray(bias))

    B = x.shape[0]
    in_maps = make_in_maps(x, core_ins)
    res = bass_utils.run_bass_kernel_spmd(nc, in_maps,
                                          core_ids=list(range(N_CORES)))

    out = np.empty((B, C, S, S), np.float32)
    for c in range(N_CORES):
        yc = res.results[c]["y"].reshape(4, NBI, 784).transpose(1, 0, 2)
        yc = yc.reshape(IL, 2, 56, 56)
        out[:, IL * c:IL * c + IL] = np.transpose(yc, (1, 0, 2, 3))
    return out


# revision 13
# speedup vs baseline: 1.0536x; 1.0536x over previous
"""DivisiveNormBlock kernel v4 for 8 Trainium2 NeuronCores.

out[b,i] = x[b,i]^nU[i] / (bias[i]^nU[i] + sum_u conv2d(x[b,i]^nI[i,u], g[i,u]))

v4 strategy: G=16 images per group with KB=8 basis nodes per channel
(GKB=128) -> only NGR=2 groups, halving both the conv matmul passes
(NGR*T2 = 12) and the Act-engine exp volume vs v2. Bias is folded into
the tap-sum matmul via a constant-ones partition row (K=97), the
reciprocal runs straight from PSUM, and the final numerator/multiply
work in the compact [128, 784] quarter-major layout. Weight DMAs and
border memsets are hoisted out of the steady-state loop; group 1's
basis passes interleave into group 0's conv to keep the PE p-state
warm. The timing loop unrolls UNROLL bodies per hardware iteration
with double-buffered head/tail tiles so consecutive iterations
software-pipeline across the loop.
"""

import math
import numpy as np
import ml_dtypes

C = 128
S = 56
KS = 6
N_CORES = 8
IL = C // N_CORES          # 16 channels per core
NBI = IL * 2               # 32 (channel, batch) images per core
WP = 64                    # padded image row pitch
IMG = 64 * WP              # 4096
SOUT = S * WP              # 3584
W2Z = 3648                 # z2 width (SOUT + 5 rounded to 64)
T1 = 6                     # residual taps (kx in {0..5})
T2 = 6                     # accumulated row-shift passes (ky in {0..5})
NEG = -1e30

KB = 8                     # basis nodes per channel
G = 16                     # images per conv group
NGR = NBI // G             # 2
GKB = G * KB               # 128
T1G = T1 * G               # 96
UNROLL = 1                 # bodies per hardware loop iteration

_cache = {}


def _gaussian_bank(theta, p, sig, a):
    K = 3
    coords = np.linspace(-K, K, 2 * K)
    xv, yv = np.meshgrid(coords, coords, indexing="ij")
    ct = np.cos(theta)[:, :, None, None]
    st = np.sin(theta)[:, :, None, None]
    xr = xv * ct + yv * st
    yr = -xv * st + yv * ct
    p2 = (p ** 2)[:, :, None, None]
    s2 = (sig ** 2)[:, :, None, None]
    amp = (a / (2.0 * np.pi * p * sig))[:, :, None, None]
    return amp * np.exp(-0.5 * (xr ** 2 / p2 + yr ** 2 / s2))   # [C,C,6,6]


def _fit_chan(nvals, kb):
    """Per-channel LS fit: e^{n l} ~ sum_k c_k e^{a_k l}, l in [-19, 0]."""
    n_lo = max(float(nvals.min()) * 0.9, 1e-4)
    n_hi = float(nvals.max()) * 1.02
    aks = np.geomspace(n_lo, n_hi, kb)
    l_grid = np.linspace(-19.0, 0.0, 4000)
    A = np.exp(np.outer(l_grid, aks))
    AtA = A.T @ A + 1e-6 * np.eye(kb)
    Y = np.exp(np.outer(l_grid, nvals))
    Cf = np.linalg.solve(AtA, A.T @ Y)               # [kb, C]
    return aks, Cf


def _build_host_params(theta, p, sig, a, nI, nU, bias, kb=KB):
    f64 = np.float64
    g = _gaussian_bank(theta.astype(f64), p.astype(f64), sig.astype(f64),
                       a.astype(f64))                 # [C,C,6,6]
    nI64 = nI.astype(f64)
    aks_all = np.zeros((C, kb))
    W2_all = np.zeros((C, kb, KS, KS))
    for i in range(C):
        aks, Cf = _fit_chan(nI64[i], kb)
        aks_all[i] = aks
        W2_all[i] = np.einsum("uyx,ku->kyx", g[i], Cf)
    biasP = bias.astype(f64) ** nU.astype(f64)
    return aks_all, W2_all, biasP


def _build_program(loop_n=None, debug=False, unroll_n=None):
    import concourse.bacc as bacc
    import concourse.mybir as mybir
    from concourse.tile import TileContext
    from contextlib import nullcontext

    f32, f32r, bf16 = mybir.dt.float32, mybir.dt.float32r, mybir.dt.bfloat16
    AF = mybir.ActivationFunctionType

    # bodies per hw-loop iteration: loop_n=k*UNROLL runs For_i(k) x UNROLL
    if unroll_n is not None:
        hw_n, nbody = None, unroll_n
    elif loop_n and loop_n % UNROLL == 0:
        hw_n, nbody = loop_n // UNROLL, UNROLL
    elif loop_n:
        hw_n, nbody = loop_n, 1
    else:
        hw_n, nbody = None, 1

    nc = bacc.Bacc("TRN2", debug=False)
    xs = nc.dram_tensor("xs", [128, 784], f32, kind="ExternalInput")
    e3 = nc.dram_tensor("e3", [NBI, NGR * GKB], f32r, kind="ExternalInput")
    w3 = nc.dram_tensor("w3", [GKB, NGR * T2 * T1G], f32r, kind="ExternalInput")
    o3 = nc.dram_tensor("o3", [T1G + 1, NGR * NBI], bf16, kind="ExternalInput")
    nUr = nc.dram_tensor("nUr", [128, 1], f32, kind="ExternalInput")
    y = nc.dram_tensor("y", [128, 784], f32, kind="ExternalOutput")

    # conv output chunks over z2 cols [0, W2Z): 7 x 512 + 1 x 64
    conv_chunks = [(512 * i, 512) for i in range(7)] + [(3584, W2Z - 3584)]

    with TileContext(nc) as tc:
        with tc.tile_pool(name="const", bufs=1) as cpool, \
             tc.tile_pool(name="pbc", bufs=2, space="PSUM") as pbc, \
             tc.tile_pool(name="pcv", bufs=2, space="PSUM") as pcv, \
             tc.tile_pool(name="pts", bufs=4, space="PSUM") as pts:
            # ---- persistent tiles; head/tail set double-buffered ----
            x_t = [cpool.tile([128, 784], f32, name=f"x_{d}") for d in (0, 1)]
            l_t = [cpool.tile([128, 784], f32, name=f"l_{d}") for d in (0, 1)]
            num_t = [cpool.tile([128, 784], f32, name=f"num_{d}")
                     for d in (0, 1)]
            rT_t = [cpool.tile([128, 784], f32, name=f"rT_{d}")
                    for d in (0, 1)]
            o_t = [cpool.tile([128, 784], f32, name=f"o_{d}") for d in (0, 1)]
            r_t = [cpool.tile([NBI, SOUT], f32, name=f"r_{d}") for d in (0, 1)]
            lp_t = cpool.tile([NBI, IMG], f32r)
            b3_t = [cpool.tile([GKB, IMG], f32r, name=f"b3_{g}")
                    for g in range(NGR)]
            z2_t = [cpool.tile([T1G, W2Z], bf16, name=f"z2_{g}")
                    for g in range(NGR)]
            z3_t = [cpool.tile([T1G + 1, SOUT], bf16, name=f"z3_{g}")
                    for g in range(NGR)]
            e3_t = cpool.tile([NBI, NGR * GKB], f32r)
            w3_t = cpool.tile([GKB, NGR * T2 * T1G], f32r)
            o3_t = cpool.tile([T1G + 1, NGR * NBI], bf16)
            nU_t = cpool.tile([128, 1], f32)

            # ---- hoisted setup: params + constant borders (once) ----
            nc.sync.dma_start(e3_t[:], e3.ap())
            nc.sync.dma_start(w3_t[:], w3.ap())
            nc.sync.dma_start(o3_t[:], o3.ap())
            nc.sync.dma_start(nU_t[:], nUr.ap())
            nc.vector.memset(lp_t[:].bitcast(f32), NEG)
            # ones row for the bias fold (partition 96 of group-0 z3)
            nc.vector.memset(z3_t[0][T1G:T1G + 1, :], 1.0)

            def body(pp):
                xt, lt, numt = x_t[pp], l_t[pp], num_t[pp]
                rt, rTt, ot = r_t[pp], rT_t[pp], o_t[pp]
                nc.sync.dma_start(xt[:], xs.ap())

                # l = clamp(ln(x)); x=0 -> -inf -> -1e30
                nc.scalar.activation(lt[:], xt[:], AF.Ln)
                nc.vector.tensor_scalar_max(lt[:], lt[:], NEG)

                # padded log images, one partition per (i,b); 4 fused DMAs
                pad_engs = (nc.sync, nc.scalar, nc.sync, nc.scalar)
                for q in range(4):
                    src = lt[32 * q:32 * q + 32, :].bitcast(f32r).rearrange(
                        "p (r c) -> p r c", c=56)
                    dst = lp_t[:].rearrange("p (r c) -> p r c", r=64)[
                        :, 2 + 14 * q:2 + 14 * q + 14, 2:58]
                    pad_engs[q].dma_start(dst, src)

                def b3_pass(g, h):
                    pb = pbc.tile([GKB, 512], f32, tag="pb")
                    nc.tensor.matmul(
                        pb[:, :],
                        e3_t[:, GKB * g:GKB * g + GKB],
                        lp_t[:, 512 * h:512 * h + 512],
                        start=True, stop=True)
                    nc.scalar.activation(
                        b3_t[g][:, 512 * h:512 * h + 512], pb[:, :], AF.Exp)

                # group 0 basis maps: b3[im*KB+k,s] = exp(aks[i_im,k]*lp[im,s])
                for h in range(IMG // 512):
                    b3_pass(0, h)

                # conv with ky-shifted reads accumulated over T2 passes:
                # z2[im*T1 + t1, s] = sum_k sum_t2 W2[i_im,k,t2,t1]
                #                     * b3[im*KB+k, s + 64*t2]
                nmix = 0

                def conv_group(g, interleave, lo=0, hi=8):
                    nonlocal nmix
                    for ci, (s0, cw) in list(enumerate(conv_chunks))[lo:hi]:
                        pc = pcv.tile([T1G, 512], f32, tag="pc")
                        for t2 in range(T2):
                            nc.tensor.matmul(
                                pc[:, 0:cw],
                                w3_t[:, T1G * (T2 * g + t2):
                                     T1G * (T2 * g + t2) + T1G],
                                b3_t[g][:, s0 + 64 * t2:s0 + 64 * t2 + cw],
                                start=(t2 == 0), stop=(t2 == T2 - 1))
                        if nmix % 2 == 0:
                            nc.vector.tensor_copy(z2_t[g][:, s0:s0 + cw],
                                                  pc[:, 0:cw])
                        else:
                            nc.scalar.copy(z2_t[g][:, s0:s0 + cw],
                                           pc[:, 0:cw])
                        nmix += 1
                        if interleave and ci < IMG // 512:
                            b3_pass(1, ci)

                def realign(g, c0=0, c1=SOUT):
                    # residual kx realign: 6 strided-partition DMAs
                    re_engs = (nc.sync, nc.gpsimd, nc.sync,
                               nc.gpsimd, nc.sync, nc.gpsimd)
                    for t1 in range(T1):
                        re_engs[t1].dma_start(
                            z3_t[g][t1:T1G:T1, c0:c1],
                            z2_t[g][t1:T1G:T1, t1 + c0:t1 + c1])

                def tap(ch):
                    # tap sum over groups (o3 cols select the group's
                    # images; row 96 of group 0 adds bias via the ones
                    # row), then the reciprocal straight from PSUM
                    sl = slice(512 * ch, 512 * ch + 512)
                    pt = pts.tile([NBI, 512], f32, tag="pt")
                    nc.tensor.matmul(
                        pt[:, :], o3_t[0:T1G + 1, 0:NBI],
                        z3_t[0][0:T1G + 1, sl],
                        start=True, stop=False, skip_group_check=True)
                    nc.tensor.matmul(
                        pt[:, :], o3_t[0:T1G, NBI:2 * NBI],
                        z3_t[1][0:T1G, sl],
                        start=False, stop=True, skip_group_check=True)
                    nc.vector.reciprocal(rt[:, sl], pt[:, :])

                tr_engs = (nc.sync, nc.scalar, nc.sync, nc.scalar)

                def transpose_q(q):
                    # inverse-pad DMA back to [128, 784] quarter-major
                    src = rt[:].rearrange("p (r c) -> p r c", c=64)[
                        :, 14 * q:14 * q + 14, 0:56]
                    dst = rTt[32 * q:32 * q + 32, :].rearrange(
                        "p (r c) -> p r c", c=56)
                    tr_engs[q].dma_start(dst, src)

                conv_group(0, interleave=True)
                realign(0)
                # numerator x^nU = exp(nU * l) on the compact layout
                nc.scalar.activation(numt[:], lt[:], AF.Exp, scale=nU_t[:])
                conv_group(1, interleave=False)
                realign(1)
                for ch in range(SOUT // 512):
                    tap(ch)
                for q in range(4):
                    transpose_q(q)
                nc.vector.tensor_mul(ot[:], numt[:], rTt[:])
                nc.sync.dma_start(y.ap(), ot[:])

            loop_ctx = tc.For_i(0, hw_n, 1) if hw_n else nullcontext()
            with loop_ctx:
                for it in range(nbody):
                    body(it % 2)

    nc.compile()
    return nc


def _get_compiled(theta, p, sig, a, nI, nU, bias):
    key = "prog"
    if key in _cache:
        return _cache[key]
    aks_all, W2_all, biasP = _build_host_params(theta, p, sig, a, nI, nU, bias)
    nc = _build_program()

    bft = ml_dtypes.bfloat16
    core_ins = []
    for c in range(N_CORES):
        i0 = IL * c
        e3 = np.zeros((NBI, NGR * GKB), np.float32)
        w3 = np.zeros((GKB, NGR * T2 * T1G), np.float32)
        o3 = np.zeros((T1G + 1, NGR * NBI), np.float32)
        for g in range(NGR):
            for im in range(G):
                bi = G * g + im
                i = i0 + bi // 2
                e3[bi, GKB * g + KB * im:GKB * g + KB * im + KB] = aks_all[i]
                for t2 in range(T2):
                    blk = T1G * (T2 * g + t2)
                    for t1 in range(T1):
                        w3[KB * im:KB * im + KB, blk + T1 * im + t1] = \
                            W2_all[i, :, t2, t1]
                o3[T1 * im:T1 * im + T1, NBI * g + bi] = 1.0
        for bi in range(NBI):
            o3[T1G, bi] = biasP[i0 + bi // 2]     # bias via group-0 ones row
        nU_rep = np.repeat(nU[i0:i0 + IL].astype(np.float32), 2)
        nU128 = np.tile(nU_rep, 4)[:, None]       # quarter-major partitions
        core_ins.append({
            "e3": np.ascontiguousarray(e3),
            "w3": np.ascontiguousarray(w3),
            "o3": np.ascontiguousarray(o3.astype(bft)),
            "nUr": np.ascontiguousarray(nU128),
        })
    _cache[key] = (nc, core_ins)
    return _cache[key]


def make_in_maps(x, core_ins):
    in_maps = []
    for c in range(N_CORES):
        i0 = IL * c
        xc = np.transpose(x[:, i0:i0 + IL], (1, 0, 2, 3))   # [16, 2, 56, 56]
        # row = q*32 + bi: quarter-major so each 14-row slab of every image
        # sits in one contiguous 32-partition block
        xs = xc.reshape(IL * 2, 4, 784).transpose(1, 0, 2).reshape(128, 784)
        in_maps.append({"xs": np.ascontiguousarray(xs.astype(np.float32)),
                        **core_ins[c]})
    return in_maps


def kernel(x, theta, p, sig, a, nI, nU, bias):
    from concourse import bass_utils

    x = np.asarray(x)
    nc, core_ins = _get_compiled(
        np.asarray(theta), np.asarray(p), np.asarray(sig), np.asarray(a),
        np.asarray(nI), np.asarray(nU), np.asarray(bias))

    B = x.shape[0]
    in_maps = make_in_maps(x, core_ins)
    res = bass_utils.run_bass_kernel_spmd(nc, in_maps,
                                          core_ids=list(range(N_CORES)))

    out = np.empty((B, C, S, S), np.float32)
    for c in range(N_CORES):
        yc = res.results[c]["y"].reshape(4, NBI, 784).transpose(1, 0, 2)
        yc = yc.reshape(IL, 2, 56, 56)
        out[:, IL * c:IL * c + IL] = np.transpose(yc, (1, 0, 2, 3))
    return out


# revision 14
# speedup vs baseline: 1.1678x; 1.1083x over previous
"""DivisiveNormBlock kernel v4 for 8 Trainium2 NeuronCores.

out[b,i] = x[b,i]^nU[i] / (bias[i]^nU[i] + sum_u conv2d(x[b,i]^nI[i,u], g[i,u]))

v4 strategy: G=16 images per group with KB=8 basis nodes per channel
(GKB=128) -> only NGR=2 groups, halving both the conv matmul passes
(NGR*T2 = 12) and the Act-engine exp volume vs v2. Bias is folded into
the tap-sum matmul via a constant-ones partition row (K=97), the
reciprocal runs straight from PSUM, and the final numerator/multiply
work in the compact [128, 784] quarter-major layout. Weight DMAs and
border memsets are hoisted out of the steady-state loop; group 1's
basis passes interleave into group 0's conv to keep the PE p-state
warm. The timing loop unrolls UNROLL bodies per hardware iteration
with double-buffered head/tail tiles so consecutive iterations
software-pipeline across the loop.
"""

import math
import numpy as np
import ml_dtypes

C = 128
S = 56
KS = 6
N_CORES = 8
IL = C // N_CORES          # 16 channels per core
NBI = IL * 2               # 32 (channel, batch) images per core
WP = 64                    # padded image row pitch
IMG = 64 * WP              # 4096
SOUT = S * WP              # 3584
W2Z = 3648                 # z2 width (SOUT + 5 rounded to 64)
T1 = 6                     # residual taps (kx in {0..5})
T2 = 6                     # accumulated row-shift passes (ky in {0..5})
NEG = -1e30

KB = 8                     # basis nodes per channel
G = 16                     # images per conv group
NGR = NBI // G             # 2
GKB = G * KB               # 128
T1G = T1 * G               # 96
UNROLL = 2                 # bodies per hardware loop iteration

_cache = {}


def _gaussian_bank(theta, p, sig, a):
    K = 3
    coords = np.linspace(-K, K, 2 * K)
    xv, yv = np.meshgrid(coords, coords, indexing="ij")
    ct = np.cos(theta)[:, :, None, None]
    st = np.sin(theta)[:, :, None, None]
    xr = xv * ct + yv * st
    yr = -xv * st + yv * ct
    p2 = (p ** 2)[:, :, None, None]
    s2 = (sig ** 2)[:, :, None, None]
    amp = (a / (2.0 * np.pi * p * sig))[:, :, None, None]
    return amp * np.exp(-0.5 * (xr ** 2 / p2 + yr ** 2 / s2))   # [C,C,6,6]


def _fit_chan(nvals, kb):
    """Per-channel LS fit: e^{n l} ~ sum_k c_k e^{a_k l}, l in [-19, 0]."""
    n_lo = max(float(nvals.min()) * 0.9, 1e-4)
    n_hi = float(nvals.max()) * 1.02
    aks = np.geomspace(n_lo, n_hi, kb)
    l_grid = np.linspace(-19.0, 0.0, 4000)
    A = np.exp(np.outer(l_grid, aks))
    AtA = A.T @ A + 1e-6 * np.eye(kb)
    Y = np.exp(np.outer(l_grid, nvals))
    Cf = np.linalg.solve(AtA, A.T @ Y)               # [kb, C]
    return aks, Cf


def _build_host_params(theta, p, sig, a, nI, nU, bias, kb=KB):
    f64 = np.float64
    g = _gaussian_bank(theta.astype(f64), p.astype(f64), sig.astype(f64),
                       a.astype(f64))                 # [C,C,6,6]
    nI64 = nI.astype(f64)
    aks_all = np.zeros((C, kb))
    W2_all = np.zeros((C, kb, KS, KS))
    for i in range(C):
        aks, Cf = _fit_chan(nI64[i], kb)
        aks_all[i] = aks
        W2_all[i] = np.einsum("uyx,ku->kyx", g[i], Cf)
    biasP = bias.astype(f64) ** nU.astype(f64)
    return aks_all, W2_all, biasP


def _build_program(loop_n=None, debug=False, unroll_n=None):
    import concourse.bacc as bacc
    import concourse.mybir as mybir
    from concourse.tile import TileContext
    from contextlib import nullcontext

    f32, f32r, bf16 = mybir.dt.float32, mybir.dt.float32r, mybir.dt.bfloat16
    AF = mybir.ActivationFunctionType

    # bodies per hw-loop iteration: loop_n=k*UNROLL runs For_i(k) x UNROLL
    if unroll_n is not None:
        hw_n, nbody = None, unroll_n
    elif loop_n and loop_n % UNROLL == 0:
        hw_n, nbody = loop_n // UNROLL, UNROLL
    elif loop_n:
        hw_n, nbody = loop_n, 1
    else:
        hw_n, nbody = None, 1

    nc = bacc.Bacc("TRN2", debug=False)
    xs = nc.dram_tensor("xs", [128, 784], f32, kind="ExternalInput")
    e3 = nc.dram_tensor("e3", [NBI, NGR * GKB], f32r, kind="ExternalInput")
    w3 = nc.dram_tensor("w3", [GKB, NGR * T2 * T1G], f32r, kind="ExternalInput")
    o3 = nc.dram_tensor("o3", [T1G + 1, NGR * NBI], bf16, kind="ExternalInput")
    nUr = nc.dram_tensor("nUr", [128, 1], f32, kind="ExternalInput")
    y = nc.dram_tensor("y", [128, 784], f32, kind="ExternalOutput")

    # conv output chunks over z2 cols [0, W2Z): 7 x 512 + 1 x 64
    conv_chunks = [(512 * i, 512) for i in range(7)] + [(3584, W2Z - 3584)]

    with TileContext(nc) as tc:
        with tc.tile_pool(name="const", bufs=1) as cpool, \
             tc.tile_pool(name="pbc", bufs=2, space="PSUM") as pbc, \
             tc.tile_pool(name="pcv", bufs=2, space="PSUM") as pcv, \
             tc.tile_pool(name="pts", bufs=4, space="PSUM") as pts:
            # ---- persistent tiles; head/tail set double-buffered ----
            x_t = [cpool.tile([128, 784], f32, name=f"x_{d}") for d in (0, 1)]
            l_t = [cpool.tile([128, 784], f32, name=f"l_{d}") for d in (0, 1)]
            num_t = [cpool.tile([128, 784], f32, name=f"num_{d}")
                     for d in (0, 1)]
            rT_t = [cpool.tile([128, 784], f32, name=f"rT_{d}")
                    for d in (0, 1)]
            o_t = [cpool.tile([128, 784], f32, name=f"o_{d}") for d in (0, 1)]
            r_t = [cpool.tile([NBI, SOUT], f32, name=f"r_{d}") for d in (0, 1)]
            lp_t = cpool.tile([NBI, IMG], f32r)
            b3_t = [cpool.tile([GKB, IMG], f32r, name=f"b3_{g}")
                    for g in range(NGR)]
            z2_t = [cpool.tile([T1G, W2Z], bf16, name=f"z2_{g}")
                    for g in range(NGR)]
            z3_t = [cpool.tile([T1G + 1, SOUT], bf16, name=f"z3_{g}")
                    for g in range(NGR)]
            e3_t = cpool.tile([NBI, NGR * GKB], f32r)
            w3_t = cpool.tile([GKB, NGR * T2 * T1G], f32r)
            o3_t = cpool.tile([T1G + 1, NGR * NBI], bf16)
            nU_t = cpool.tile([128, 1], f32)

            # ---- hoisted setup: params + constant borders (once) ----
            nc.sync.dma_start(e3_t[:], e3.ap())
            nc.sync.dma_start(w3_t[:], w3.ap())
            nc.sync.dma_start(o3_t[:], o3.ap())
            nc.sync.dma_start(nU_t[:], nUr.ap())
            nc.vector.memset(lp_t[:].bitcast(f32), NEG)
            # ones row for the bias fold (partition 96 of group-0 z3)
            nc.vector.memset(z3_t[0][T1G:T1G + 1, :], 1.0)

            def body(pp):
                xt, lt, numt = x_t[pp], l_t[pp], num_t[pp]
                rt, rTt, ot = r_t[pp], rT_t[pp], o_t[pp]
                nc.sync.dma_start(xt[:], xs.ap())

                # l = clamp(ln(x)); x=0 -> -inf -> -1e30
                nc.scalar.activation(lt[:], xt[:], AF.Ln)
                nc.vector.tensor_scalar_max(lt[:], lt[:], NEG)

                # padded log images, one partition per (i,b); 4 fused DMAs
                pad_engs = (nc.sync, nc.scalar, nc.sync, nc.scalar)
                for q in range(4):
                    src = lt[32 * q:32 * q + 32, :].bitcast(f32r).rearrange(
                        "p (r c) -> p r c", c=56)
                    dst = lp_t[:].rearrange("p (r c) -> p r c", r=64)[
                        :, 2 + 14 * q:2 + 14 * q + 14, 2:58]
                    pad_engs[q].dma_start(dst, src)

                def b3_pass(g, h):
                    pb = pbc.tile([GKB, 512], f32, tag="pb")
                    nc.tensor.matmul(
                        pb[:, :],
                        e3_t[:, GKB * g:GKB * g + GKB],
                        lp_t[:, 512 * h:512 * h + 512],
                        start=True, stop=True)
                    nc.scalar.activation(
                        b3_t[g][:, 512 * h:512 * h + 512], pb[:, :], AF.Exp)

                # group 0 basis maps: b3[im*KB+k,s] = exp(aks[i_im,k]*lp[im,s])
                for h in range(IMG // 512):
                    b3_pass(0, h)

                # conv with ky-shifted reads accumulated over T2 passes:
                # z2[im*T1 + t1, s] = sum_k sum_t2 W2[i_im,k,t2,t1]
                #                     * b3[im*KB+k, s + 64*t2]
                nmix = 0

                def conv_group(g, interleave, lo=0, hi=8):
                    nonlocal nmix
                    for ci, (s0, cw) in list(enumerate(conv_chunks))[lo:hi]:
                        pc = pcv.tile([T1G, 512], f32, tag="pc")
                        for t2 in range(T2):
                            nc.tensor.matmul(
                                pc[:, 0:cw],
                                w3_t[:, T1G * (T2 * g + t2):
                                     T1G * (T2 * g + t2) + T1G],
                                b3_t[g][:, s0 + 64 * t2:s0 + 64 * t2 + cw],
                                start=(t2 == 0), stop=(t2 == T2 - 1))
                        if nmix % 2 == 0:
                            nc.vector.tensor_copy(z2_t[g][:, s0:s0 + cw],
                                                  pc[:, 0:cw])
                        else:
                            nc.scalar.copy(z2_t[g][:, s0:s0 + cw],
                                           pc[:, 0:cw])
                        nmix += 1
                        if interleave and ci < IMG // 512:
                            b3_pass(1, ci)

                def realign(g, c0=0, c1=SOUT):
                    # residual kx realign: 6 strided-partition DMAs
                    re_engs = (nc.sync, nc.gpsimd, nc.sync,
                               nc.gpsimd, nc.sync, nc.gpsimd)
                    for t1 in range(T1):
                        re_engs[t1].dma_start(
                            z3_t[g][t1:T1G:T1, c0:c1],
                            z2_t[g][t1:T1G:T1, t1 + c0:t1 + c1])

                def tap(ch):
                    # tap sum over groups (o3 cols select the group's
                    # images; row 96 of group 0 adds bias via the ones
                    # row), then the reciprocal straight from PSUM
                    sl = slice(512 * ch, 512 * ch + 512)
                    pt = pts.tile([NBI, 512], f32, tag="pt")
                    nc.tensor.matmul(
                        pt[:, :], o3_t[0:T1G + 1, 0:NBI],
                        z3_t[0][0:T1G + 1, sl],
                        start=True, stop=False, skip_group_check=True)
                    nc.tensor.matmul(
                        pt[:, :], o3_t[0:T1G, NBI:2 * NBI],
                        z3_t[1][0:T1G, sl],
                        start=False, stop=True, skip_group_check=True)
                    nc.vector.reciprocal(rt[:, sl], pt[:, :])

                tr_engs = (nc.sync, nc.scalar, nc.sync, nc.scalar)

                def transpose_q(q):
                    # inverse-pad DMA back to [128, 784] quarter-major
                    src = rt[:].rearrange("p (r c) -> p r c", c=64)[
                        :, 14 * q:14 * q + 14, 0:56]
                    dst = rTt[32 * q:32 * q + 32, :].rearrange(
                        "p (r c) -> p r c", c=56)
                    tr_engs[q].dma_start(dst, src)

                conv_group(0, interleave=True)
                realign(0)
                # numerator x^nU = exp(nU * l) on the compact layout
                nc.scalar.activation(numt[:], lt[:], AF.Exp, scale=nU_t[:])
                conv_group(1, interleave=False)
                realign(1)
                for ch in range(SOUT // 512):
                    tap(ch)
                for q in range(4):
                    transpose_q(q)
                nc.vector.tensor_mul(ot[:], numt[:], rTt[:])
                nc.sync.dma_start(y.ap(), ot[:])

            loop_ctx = tc.For_i(0, hw_n, 1) if hw_n else nullcontext()
            with loop_ctx:
                for it in range(nbody):
                    body(it % 2)

    nc.compile()
    return nc


def _get_compiled(theta, p, sig, a, nI, nU, bias):
    key = "prog"
    if key in _cache:
        return _cache[key]
    aks_all, W2_all, biasP = _build_host_params(theta, p, sig, a, nI, nU, bias)
    nc = _build_program()

    bft = ml_dtypes.bfloat16
    core_ins = []
    for c in range(N_CORES):
        i0 = IL * c
        e3 = np.zeros((NBI, NGR * GKB), np.float32)
        w3 = np.zeros((GKB, NGR * T2 * T1G), np.float32)
        o3 = np.zeros((T1G + 1, NGR * NBI), np.float32)
        for g in range(NGR):
            for im in range(G):
                bi = G * g + im
                i = i0 + bi // 2
                e3[bi, GKB * g + KB * im:GKB * g + KB * im + KB] = aks_all[i]
                for t2 in range(T2):
                    blk = T1G * (T2 * g + t2)
                    for t1 in range(T1):
                        w3[KB * im:KB * im + KB, blk + T1 * im + t1] = \
                            W2_all[i, :, t2, t1]
                o3[T1 * im:T1 * im + T1, NBI * g + bi] = 1.0
        for bi in range(NBI):
            o3[T1G, bi] = biasP[i0 + bi // 2]     # bias via group-0 ones row
        nU_rep = np.repeat(nU[i0:i0 + IL].astype(np.float32), 2)
        nU128 = np.tile(nU_rep, 4)[:, None]       # quarter-major partitions
        core_ins.append({
            "e3": np.ascontiguousarray(e3),
            "w3": np.ascontiguousarray(w3),
            "o3": np.ascontiguousarray(o3.astype(bft)),
            "nUr": np.ascontiguousarray(nU128),
        })
    _cache[key] = (nc, core_ins)
    return _cache[key]


def make_in_maps(x, core_ins):
    in_maps = []
    for c in range(N_CORES):
        i0 = IL * c
        xc = np.transpose(x[:, i0:i0 + IL], (1, 0, 2, 3))   # [16, 2, 56, 56]
        # row = q*32 + bi: quarter-major so each 14-row slab of every image
        # sits in one contiguous 32-partition block
        xs = xc.reshape(IL * 2, 4, 784).transpose(1, 0, 2).reshape(128, 784)
        in_maps.append({"xs": np.ascontiguousarray(xs.astype(np.float32)),
                        **core_ins[c]})
    return in_maps


def kernel(x, theta, p, sig, a, nI, nU, bias):
    from concourse import bass_utils

    x = np.asarray(x)
    nc, core_ins = _get_compiled(
        np.asarray(theta), np.asarray(p), np.asarray(sig), np.asarray(a),
        np.asarray(nI), np.asarray(nU), np.asarray(bias))

    B = x.shape[0]
    in_maps = make_in_maps(x, core_ins)
    res = bass_utils.run_bass_kernel_spmd(nc, in_maps,
                                          core_ids=list(range(N_CORES)))

    out = np.empty((B, C, S, S), np.float32)
    for c in range(N_CORES):
        yc = res.results[c]["y"].reshape(4, NBI, 784).transpose(1, 0, 2)
        yc = yc.reshape(IL, 2, 56, 56)
        out[:, IL * c:IL * c + IL] = np.transpose(yc, (1, 0, 2, 3))
    return out


# revision 15
# speedup vs baseline: 1.1927x; 1.0214x over previous
"""DivisiveNormBlock kernel v4 for 8 Trainium2 NeuronCores.

out[b,i] = x[b,i]^nU[i] / (bias[i]^nU[i] + sum_u conv2d(x[b,i]^nI[i,u], g[i,u]))

v4 strategy: G=16 images per group with KB=8 basis nodes per channel
(GKB=128) -> only NGR=2 groups, halving both the conv matmul passes
(NGR*T2 = 12) and the Act-engine exp volume vs v2. Bias is folded into
the tap-sum matmul via a constant-ones partition row (K=97), the
reciprocal runs straight from PSUM, and the final numerator/multiply
work in the compact [128, 784] quarter-major layout. Weight DMAs and
border memsets are hoisted out of the steady-state loop; group 1's
basis passes interleave into group 0's conv to keep the PE p-state
warm. The timing loop unrolls UNROLL bodies per hardware iteration
with double-buffered head/tail tiles so consecutive iterations
software-pipeline across the loop.
"""

import math
import numpy as np
import ml_dtypes

C = 128
S = 56
KS = 6
N_CORES = 8
IL = C // N_CORES          # 16 channels per core
NBI = IL * 2               # 32 (channel, batch) images per core
WP = 64                    # padded image row pitch
IMG = 64 * WP              # 4096
SOUT = S * WP              # 3584
W2Z = 3648                 # z2 width (SOUT + 5 rounded to 64)
T1 = 6                     # residual taps (kx in {0..5})
T2 = 6                     # accumulated row-shift passes (ky in {0..5})
NEG = -1e30

KB = 8                     # basis nodes per channel
G = 16                     # images per conv group
NGR = NBI // G             # 2
GKB = G * KB               # 128
T1G = T1 * G               # 96
UNROLL = 4                 # bodies per hardware loop iteration

_cache = {}


def _gaussian_bank(theta, p, sig, a):
    K = 3
    coords = np.linspace(-K, K, 2 * K)
    xv, yv = np.meshgrid(coords, coords, indexing="ij")
    ct = np.cos(theta)[:, :, None, None]
    st = np.sin(theta)[:, :, None, None]
    xr = xv * ct + yv * st
    yr = -xv * st + yv * ct
    p2 = (p ** 2)[:, :, None, None]
    s2 = (sig ** 2)[:, :, None, None]
    amp = (a / (2.0 * np.pi * p * sig))[:, :, None, None]
    return amp * np.exp(-0.5 * (xr ** 2 / p2 + yr ** 2 / s2))   # [C,C,6,6]


def _fit_chan(nvals, kb):
    """Per-channel LS fit: e^{n l} ~ sum_k c_k e^{a_k l}, l in [-19, 0]."""
    n_lo = max(float(nvals.min()) * 0.9, 1e-4)
    n_hi = float(nvals.max()) * 1.02
    aks = np.geomspace(n_lo, n_hi, kb)
    l_grid = np.linspace(-19.0, 0.0, 4000)
    A = np.exp(np.outer(l_grid, aks))
    AtA = A.T @ A + 1e-6 * np.eye(kb)
    Y = np.exp(np.outer(l_grid, nvals))
    Cf = np.linalg.solve(AtA, A.T @ Y)               # [kb, C]
    return aks, Cf


def _build_host_params(theta, p, sig, a, nI, nU, bias, kb=KB):
    f64 = np.float64
    g = _gaussian_bank(theta.astype(f64), p.astype(f64), sig.astype(f64),
                       a.astype(f64))                 # [C,C,6,6]
    nI64 = nI.astype(f64)
    aks_all = np.zeros((C, kb))
    W2_all = np.zeros((C, kb, KS, KS))
    for i in range(C):
        aks, Cf = _fit_chan(nI64[i], kb)
        aks_all[i] = aks
        W2_all[i] = np.einsum("uyx,ku->kyx", g[i], Cf)
    biasP = bias.astype(f64) ** nU.astype(f64)
    return aks_all, W2_all, biasP


def _build_program(loop_n=None, debug=False, unroll_n=None):
    import concourse.bacc as bacc
    import concourse.mybir as mybir
    from concourse.tile import TileContext
    from contextlib import nullcontext

    f32, f32r, bf16 = mybir.dt.float32, mybir.dt.float32r, mybir.dt.bfloat16
    AF = mybir.ActivationFunctionType

    # bodies per hw-loop iteration: loop_n=k*UNROLL runs For_i(k) x UNROLL
    if unroll_n is not None:
        hw_n, nbody = None, unroll_n
    elif loop_n and loop_n % UNROLL == 0:
        hw_n, nbody = loop_n // UNROLL, UNROLL
    elif loop_n:
        hw_n, nbody = loop_n, 1
    else:
        hw_n, nbody = None, 1

    nc = bacc.Bacc("TRN2", debug=False)
    xs = nc.dram_tensor("xs", [128, 784], f32, kind="ExternalInput")
    e3 = nc.dram_tensor("e3", [NBI, NGR * GKB], f32r, kind="ExternalInput")
    w3 = nc.dram_tensor("w3", [GKB, NGR * T2 * T1G], f32r, kind="ExternalInput")
    o3 = nc.dram_tensor("o3", [T1G + 1, NGR * NBI], bf16, kind="ExternalInput")
    nUr = nc.dram_tensor("nUr", [128, 1], f32, kind="ExternalInput")
    y = nc.dram_tensor("y", [128, 784], f32, kind="ExternalOutput")

    # conv output chunks over z2 cols [0, W2Z): 7 x 512 + 1 x 64
    conv_chunks = [(512 * i, 512) for i in range(7)] + [(3584, W2Z - 3584)]

    with TileContext(nc) as tc:
        with tc.tile_pool(name="const", bufs=1) as cpool, \
             tc.tile_pool(name="pbc", bufs=2, space="PSUM") as pbc, \
             tc.tile_pool(name="pcv", bufs=2, space="PSUM") as pcv, \
             tc.tile_pool(name="pts", bufs=4, space="PSUM") as pts:
            # ---- persistent tiles; head/tail set double-buffered ----
            x_t = [cpool.tile([128, 784], f32, name=f"x_{d}") for d in (0, 1)]
            l_t = [cpool.tile([128, 784], f32, name=f"l_{d}") for d in (0, 1)]
            num_t = [cpool.tile([128, 784], f32, name=f"num_{d}")
                     for d in (0, 1)]
            rT_t = [cpool.tile([128, 784], f32, name=f"rT_{d}")
                    for d in (0, 1)]
            o_t = [cpool.tile([128, 784], f32, name=f"o_{d}") for d in (0, 1)]
            r_t = [cpool.tile([NBI, SOUT], f32, name=f"r_{d}") for d in (0, 1)]
            lp_t = cpool.tile([NBI, IMG], f32r)
            b3_t = [cpool.tile([GKB, IMG], f32r, name=f"b3_{g}")
                    for g in range(NGR)]
            z2_t = [cpool.tile([T1G, W2Z], bf16, name=f"z2_{g}")
                    for g in range(NGR)]
            z3_t = [cpool.tile([T1G + 1, SOUT], bf16, name=f"z3_{g}")
                    for g in range(NGR)]
            e3_t = cpool.tile([NBI, NGR * GKB], f32r)
            w3_t = cpool.tile([GKB, NGR * T2 * T1G], f32r)
            o3_t = cpool.tile([T1G + 1, NGR * NBI], bf16)
            nU_t = cpool.tile([128, 1], f32)

            # ---- hoisted setup: params + constant borders (once) ----
            nc.sync.dma_start(e3_t[:], e3.ap())
            nc.sync.dma_start(w3_t[:], w3.ap())
            nc.sync.dma_start(o3_t[:], o3.ap())
            nc.sync.dma_start(nU_t[:], nUr.ap())
            nc.vector.memset(lp_t[:].bitcast(f32), NEG)
            # ones row for the bias fold (partition 96 of group-0 z3)
            nc.vector.memset(z3_t[0][T1G:T1G + 1, :], 1.0)

            def body(pp):
                xt, lt, numt = x_t[pp], l_t[pp], num_t[pp]
                rt, rTt, ot = r_t[pp], rT_t[pp], o_t[pp]
                nc.sync.dma_start(xt[:], xs.ap())

                # l = clamp(ln(x)); x=0 -> -inf -> -1e30
                nc.scalar.activation(lt[:], xt[:], AF.Ln)
                nc.vector.tensor_scalar_max(lt[:], lt[:], NEG)

                # padded log images, one partition per (i,b); 4 fused DMAs
                pad_engs = (nc.sync, nc.scalar, nc.sync, nc.scalar)
                for q in range(4):
                    src = lt[32 * q:32 * q + 32, :].bitcast(f32r).rearrange(
                        "p (r c) -> p r c", c=56)
                    dst = lp_t[:].rearrange("p (r c) -> p r c", r=64)[
                        :, 2 + 14 * q:2 + 14 * q + 14, 2:58]
                    pad_engs[q].dma_start(dst, src)

                def b3_pass(g, h):
                    pb = pbc.tile([GKB, 512], f32, tag="pb")
                    nc.tensor.matmul(
                        pb[:, :],
                        e3_t[:, GKB * g:GKB * g + GKB],
                        lp_t[:, 512 * h:512 * h + 512],
                        start=True, stop=True)
                    nc.scalar.activation(
                        b3_t[g][:, 512 * h:512 * h + 512], pb[:, :], AF.Exp)

                # group 0 basis maps: b3[im*KB+k,s] = exp(aks[i_im,k]*lp[im,s])
                for h in range(IMG // 512):
                    b3_pass(0, h)

                # conv with ky-shifted reads accumulated over T2 passes:
                # z2[im*T1 + t1, s] = sum_k sum_t2 W2[i_im,k,t2,t1]
                #                     * b3[im*KB+k, s + 64*t2]
                nmix = 0

                def conv_group(g, interleave, lo=0, hi=8):
                    nonlocal nmix
                    for ci, (s0, cw) in list(enumerate(conv_chunks))[lo:hi]:
                        pc = pcv.tile([T1G, 512], f32, tag="pc")
                        for t2 in range(T2):
                            nc.tensor.matmul(
                                pc[:, 0:cw],
                                w3_t[:, T1G * (T2 * g + t2):
                                     T1G * (T2 * g + t2) + T1G],
                                b3_t[g][:, s0 + 64 * t2:s0 + 64 * t2 + cw],
                                start=(t2 == 0), stop=(t2 == T2 - 1))
                        if nmix % 2 == 0:
                            nc.vector.tensor_copy(z2_t[g][:, s0:s0 + cw],
                                                  pc[:, 0:cw])
                        else:
                            nc.scalar.copy(z2_t[g][:, s0:s0 + cw],
                                           pc[:, 0:cw])
                        nmix += 1
                        if interleave and ci < IMG // 512:
                            b3_pass(1, ci)

                def realign(g, c0=0, c1=SOUT):
                    # residual kx realign: 6 strided-partition DMAs
                    re_engs = (nc.sync, nc.gpsimd, nc.sync,
                               nc.gpsimd, nc.sync, nc.gpsimd)
                    for t1 in range(T1):
                        re_engs[t1].dma_start(
                            z3_t[g][t1:T1G:T1, c0:c1],
                            z2_t[g][t1:T1G:T1, t1 + c0:t1 + c1])

                def tap(ch):
                    # tap sum over groups (o3 cols select the group's
                    # images; row 96 of group 0 adds bias via the ones
                    # row), then the reciprocal straight from PSUM
                    sl = slice(512 * ch, 512 * ch + 512)
                    pt = pts.tile([NBI, 512], f32, tag="pt")
                    nc.tensor.matmul(
                        pt[:, :], o3_t[0:T1G + 1, 0:NBI],
                        z3_t[0][0:T1G + 1, sl],
                        start=True, stop=False, skip_group_check=True)
                    nc.tensor.matmul(
                        pt[:, :], o3_t[0:T1G, NBI:2 * NBI],
                        z3_t[1][0:T1G, sl],
                        start=False, stop=True, skip_group_check=True)
                    nc.vector.reciprocal(rt[:, sl], pt[:, :])

                tr_engs = (nc.sync, nc.scalar, nc.sync, nc.scalar)

                def transpose_q(q):
                    # inverse-pad DMA back to [128, 784] quarter-major
                    src = rt[:].rearrange("p (r c) -> p r c", c=64)[
                        :, 14 * q:14 * q + 14, 0:56]
                    dst = rTt[32 * q:32 * q + 32, :].rearrange(
                        "p (r c) -> p r c", c=56)
                    tr_engs[q].dma_start(dst, src)

                conv_group(0, interleave=True)
                realign(0)
                # numerator x^nU = exp(nU * l) on the compact layout
                nc.scalar.activation(numt[:], lt[:], AF.Exp, scale=nU_t[:])
                conv_group(1, interleave=False)
                realign(1)
                for ch in range(SOUT // 512):
                    tap(ch)
                for q in range(4):
                    transpose_q(q)
                nc.vector.tensor_mul(ot[:], numt[:], rTt[:])
                nc.sync.dma_start(y.ap(), ot[:])

            loop_ctx = tc.For_i(0, hw_n, 1) if hw_n else nullcontext()
            with loop_ctx:
                for it in range(nbody):
                    body(it % 2)

    nc.compile()
    return nc


def _get_compiled(theta, p, sig, a, nI, nU, bias):
    key = "prog"
    if key in _cache:
        return _cache[key]
    aks_all, W2_all, biasP = _build_host_params(theta, p, sig, a, nI, nU, bias)
    nc = _build_program()

    bft = ml_dtypes.bfloat16
    core_ins = []
    for c in range(N_CORES):
        i0 = IL * c
        e3 = np.zeros((NBI, NGR * GKB), np.float32)
        w3 = np.zeros((GKB, NGR * T2 * T1G), np.float32)
        o3 = np.zeros((T1G + 1, NGR * NBI), np.float32)
        for g in range(NGR):
            for im in range(G):
                bi = G * g + im
                i = i0 + bi // 2
                e3[bi, GKB * g + KB * im:GKB * g + KB * im + KB] = aks_all[i]
                for t2 in range(T2):
                    blk = T1G * (T2 * g + t2)
                    for t1 in range(T1):
                        w3[KB * im:KB * im + KB, blk + T1 * im + t1] = \
                            W2_all[i, :, t2, t1]
                o3[T1 * im:T1 * im + T1, NBI * g + bi] = 1.0
        for bi in range(NBI):
            o3[T1G, bi] = biasP[i0 + bi // 2]     # bias via group-0 ones row
        nU_rep = np.repeat(nU[i0:i0 + IL].astype(np.float32), 2)
        nU128 = np.tile(nU_rep, 4)[:, None]       # quarter-major partitions
        core_ins.append({
            "e3": np.ascontiguousarray(e3),
            "w3": np.ascontiguousarray(w3),
            "o3": np.ascontiguousarray(o3.astype(bft)),
            "nUr": np.ascontiguousarray(nU128),
        })
    _cache[key] = (nc, core_ins)
    return _cache[key]


def make_in_maps(x, core_ins):
    in_maps = []
    for c in range(N_CORES):
        i0 = IL * c
        xc = np.transpose(x[:, i0:i0 + IL], (1, 0, 2, 3))   # [16, 2, 56, 56]
        # row = q*32 + bi: quarter-major so each 14-row slab of every image
        # sits in one contiguous 32-partition block
        xs = xc.reshape(IL * 2, 4, 784).transpose(1, 0, 2).reshape(128, 784)
        in_maps.append({"xs": np.ascontiguousarray(xs.astype(np.float32)),
                        **core_ins[c]})
    return in_maps


def kernel(x, theta, p, sig, a, nI, nU, bias):
    from concourse import bass_utils

    x = np.asarray(x)
    nc, core_ins = _get_compiled(
        np.asarray(theta), np.asarray(p), np.asarray(sig), np.asarray(a),
        np.asarray(nI), np.asarray(nU), np.asarray(bias))

    B = x.shape[0]
    in_maps = make_in_maps(x, core_ins)
    res = bass_utils.run_bass_kernel_spmd(nc, in_maps,
                                          core_ids=list(range(N_CORES)))

    out = np.empty((B, C, S, S), np.float32)
    for c in range(N_CORES):
        yc = res.results[c]["y"].reshape(4, NBI, 784).transpose(1, 0, 2)
        yc = yc.reshape(IL, 2, 56, 56)
        out[:, IL * c:IL * c + IL] = np.transpose(yc, (1, 0, 2, 3))
    return out
